# revision 69
# baseline (speedup 1.0000x reference)
"""GNN unpool (gather by clique id + scatter-add by node id) on 8 trn2 cores.

Problem: inputs [B=16, C*NC], node_ids/clique_ids [M], output [B, N*C] where
  pooled = inputs.reshape(B, C, NC)
  out[b, c, node_ids[m]] += pooled[b, c, clique_ids[m]]  for each m

Sharding: NODE ranges across 8 cores (each core owns ~N/8 nodes and the
~M/8 membership entries that target them). Every core holds the full
pooled tensor, staged by the host already transposed to poolT [NC, B*C]
fp8-e3m4, so the per-entry gather moves one 1KB row per entry (8x fewer
descriptors than batch sharding -> the SWDGE Q7 descriptor-generation
loop, ~8ns/index, and the sub-512B DMA penalty both drop ~8x; fp8
halves gather HBM traffic vs bf16 at a deterministic rel err of 1.6e-2
vs the 2e-2 gate).

Per-core device algorithm (uniform across cores; all per-core variation
lives in data tables so one SPMD program serves all 8):
  host packs the core's sorted entries into "slots": <=128 consecutive
  nodes and <=256 entries per slot -> exactly 2 chunks of 128 entry
  slots each (pad entries point at row 0 with an all-zero one-hot row);
  entries within a slot are ordered by clique id so gather reads are
  monotone in HBM address.
  1. dma_gather 1KB fp8 poolT rows for each chunk entry -> token layout
     ut[entry%128, chunk, B*C], groups alternating between two SWDGE
     queues so consecutive drains overlap
  2. one-hot H[entry, local-node] tables ([128, 128] fp8 per chunk) are
     compile-time known and staged from HBM directly (1.7MB) -- no
     on-device build, which kept DVE off the critical path and removed
     its SBUF-port contention with the Q7 descriptor-generation loop
  3. per slot: PE matmul psum[node 128, bc] += H_c^T @ U_c over the
     slot's 2 chunks (H is the stationary operand; 2 PSUM banks/slot)
  4. ACT/DVE (alternating per slot) evacuate psum -> bf16 staging,
     DMA -> outT[slot*128 rows, bc]
Host unshards: concatenate valid slot rows -> outT [N, B*C] -> final
[B, C*N] transpose (pure layout + dtype upcast).
"""

import math
import sys

import numpy as np

sys.path.insert(0, "/opt/trn_rl_repo")

import ml_dtypes  # noqa: E402

from concourse import bacc, mybir, tile  # noqa: E402
from concourse.bass_utils import run_bass_kernel_spmd  # noqa: E402

P = 128
N_CORES = 8
ENT_PER_SLOT = 256  # 2 chunks of 128
GSZ_SLOTS = 10  # slots per gather group (middle groups)


# ---------------------------------------------------------------- host planning


def _plan(node_ids, clique_ids, N, NC, B, C):
    node_ids = np.asarray(node_ids).astype(np.int64)
    clique_ids = np.asarray(clique_ids).astype(np.int64)
    M = node_ids.shape[0]
    bc = B * C

    order = np.argsort(node_ids, kind="stable")
    snode = node_ids[order]
    sclq = clique_ids[order]
    deg = np.bincount(node_ids, minlength=N)
    cum = np.cumsum(deg)

    # per-core contiguous node ranges, balanced by entry count
    bounds = [0]
    for d in range(1, N_CORES):
        n = int(np.searchsorted(cum, d * M / N_CORES))
        bounds.append(min(n + 1, N))
    bounds.append(N)

    cores = []
    for d in range(N_CORES):
        n0, n1 = bounds[d], bounds[d + 1]
        # greedy slots: <=128 nodes, <=256 entries, nodes atomic
        slots = []  # (g0, n_nodes, e_lo, e_hi) with e offsets into sorted arrays
        g0 = n0
        nodes_in = 0
        ents_in = 0
        for n in range(n0, n1):
            dn = int(deg[n])
            assert dn <= ENT_PER_SLOT
            if nodes_in + 1 > P or ents_in + dn > ENT_PER_SLOT:
                slots.append((g0, nodes_in, ents_in))
                g0 = n
                nodes_in, ents_in = 0, 0
            nodes_in += 1
            ents_in += dn
        if nodes_in:
            slots.append((g0, nodes_in, ents_in))
        cores.append((n0, n1, slots))

    n_slots = max(len(c[2]) for c in cores)
    n_chunks = 2 * n_slots

    core_tables = []
    for d in range(N_CORES):
        n0, n1, slots = cores[d]
        ent_clq = np.zeros(n_chunks * P, np.int16)
        nidrel_flat = np.full(n_chunks * P, -1.0, np.float32)
        e = int(np.searchsorted(snode, n0))
        slot_meta = []
        for s, (g0, nn, ne) in enumerate(slots):
            # order the slot's entries by clique id so the gather's HBM reads
            # are monotone in address (H absorbs any within-chunk permutation)
            sl_clq = sclq[e : e + ne]
            sl_rel = (snode[e : e + ne] - g0).astype(np.float32)
            o = np.argsort(sl_clq, kind="stable")
            idx = np.arange(ne)
            base = 2 * s * P
            ent_clq[base + idx] = sl_clq[o].astype(np.int16)
            nidrel_flat[base + idx] = sl_rel[o]
            slot_meta.append((g0, nn))
            e += ne
        # wrap by 16 partitions, replicate to 128 (dma_gather convention)
        wrapped = ent_clq.reshape(-1, 16).T  # [16, n_chunks*8]
        idx_tbl = np.ascontiguousarray(np.tile(wrapped, (8, 1)))
        # one-hot H staged directly as fp8 data (compile-time known):
        # H[entry e, chunk c, local node j] = 1 iff entry's node == slot
        # node j; layout [128 partitions, n_chunks*128 cols]
        h_all = np.zeros((P, n_chunks * P), ml_dtypes.float8_e3m4)
        pos = np.nonzero(nidrel_flat >= 0)[0]
        cc = pos // P
        ee = pos % P
        vv = nidrel_flat[pos].astype(np.int64)
        h_all[ee, cc * P + vv] = 1.0
        core_tables.append(
            dict(idx_tbl=idx_tbl, h_all=h_all, slot_meta=slot_meta, n0=n0, n1=n1)
        )

    return dict(
        M=M,
        N=N,
        NC=NC,
        bc=bc,
        n_slots=n_slots,
        n_chunks=n_chunks,
        cores=core_tables,
    )


# ---------------------------------------------------------------- device build


def _build(plan):
    NC = plan["NC"]
    bc = plan["bc"]
    n_slots = plan["n_slots"]
    n_chunks = plan["n_chunks"]
    assert bc % 1024 == 0 or bc in (512, 1024)
    half = bc // 2

    f32 = mybir.dt.float32
    bf16 = mybir.dt.bfloat16
    f16 = mybir.dt.float16
    f8 = mybir.dt.float8e3  # e3m4
    i16 = mybir.dt.int16

    # group schedule: small leading groups so the drain/compute pipeline
    # fills early, GSZ_SLOTS-sized middle, tapered tail so the final
    # drain+compute+write chain is short
    sizes = []
    for sz in (2, 4):
        if sum(sizes) + sz <= n_slots:
            sizes.append(sz)
    while n_slots - sum(sizes) > GSZ_SLOTS + 4:
        sizes.append(GSZ_SLOTS)
    rem = n_slots - sum(sizes)
    if rem > 4:
        sizes += [rem - 3, 2, 1]
    elif rem > 0:
        sizes.append(rem)
    group_bounds = []
    s = 0
    for z in sizes:
        group_bounds.append((s, s + z))
        s += z
    assert s == n_slots
    n_groups = len(group_bounds)

    nc = bacc.Bacc(None, target_bir_lowering=False, num_swdge_queues=2)

    poolT_d = nc.dram_tensor("poolT", [NC, bc], f8, kind="ExternalInput")
    idx_d = nc.dram_tensor("idxtbl", [P, n_chunks * 8], i16, kind="ExternalInput")
    h_d = nc.dram_tensor("htbl", [P, n_chunks * P], f8, kind="ExternalInput")
    out_d = nc.dram_tensor("out", [n_slots * P, bc], bf16, kind="ExternalOutput")

    with tile.TileContext(nc) as tc:
        with (
            tc.tile_pool(name="const", bufs=1) as constp,
            tc.tile_pool(name="upool", bufs=4) as upool,
            tc.tile_pool(name="opsum", bufs=4, space="PSUM") as opsum,
            tc.tile_pool(name="stage", bufs=6) as stagep,
        ):
            # load the first group's gather indices first so descgen can
            # start as early as possible, then H (one-hots), then the rest
            head_cols = 8 * 2 * sizes[0]
            h_head = P * 2 * sum(sizes[:2])
            idx_t = constp.tile([P, n_chunks * 8], i16)
            nc.sync.dma_start(idx_t[:, :head_cols], idx_d[:, :head_cols])
            h_t = constp.tile([P, n_chunks * P], f8)
            nc.sync.dma_start(h_t[:, :h_head], h_d[:, :h_head])
            nc.sync.dma_start(idx_t[:, head_cols:], idx_d[:, head_cols:])
            nc.sync.dma_start(h_t[:, h_head:], h_d[:, h_head:])

            u_tiles = {}

            def ensure_gather(g):
                if g in u_tiles or g >= n_groups:
                    return
                s0, s1 = group_bounds[g]
                nch = 2 * (s1 - s0)
                c0 = 2 * s0
                ut = upool.tile([P, 2 * GSZ_SLOTS, bc], f8, tag="utok")
                nidx = nch * P
                nc.gpsimd.dma_gather(
                    out_ap=ut[:, :nch, :],
                    in_ap=poolT_d[:],
                    idxs_ap=idx_t[:, c0 * 8 : (c0 + nch) * 8],
                    num_idxs=nidx,
                    num_idxs_reg=nidx,
                    elem_size=bc,
                    single_packet=False,
                    queue_num=g % 2,
                )
                u_tiles[g] = ut

            slot_group = {}
            for g, (s0, s1) in enumerate(group_bounds):
                for s in range(s0, s1):
                    slot_group[s] = g

            for s in range(n_slots):
                g = slot_group[s]
                ensure_gather(g)
                ensure_gather(g + 1)
                ut = u_tiles[g]
                la = 2 * (s - group_bounds[g][0])  # local chunk index in ut
                h0 = h_t[:, (2 * s) * P : (2 * s + 1) * P]
                h1 = h_t[:, (2 * s + 1) * P : (2 * s + 2) * P]

                pt = opsum.tile([P, bc], f32, tag="ps")  # 2 PSUM banks
                nc.tensor.matmul(
                    out=pt[:, :half],
                    lhsT=h0,
                    rhs=ut[:, la, :half],
                    start=True,
                    stop=False,
                )
                nc.tensor.matmul(
                    out=pt[:, half:],
                    lhsT=h0,
                    rhs=ut[:, la, half:],
                    start=True,
                    stop=False,
                )
                nc.tensor.matmul(
                    out=pt[:, :half],
                    lhsT=h1,
                    rhs=ut[:, la + 1, :half],
                    start=False,
                    stop=True,
                )
                nc.tensor.matmul(
                    out=pt[:, half:],
                    lhsT=h1,
                    rhs=ut[:, la + 1, half:],
                    start=False,
                    stop=True,
                )

                st = stagep.tile([P, bc], bf16, tag="st")
                # alternate evac engine: DVE is idle now that H comes from HBM
                if s % 2:
                    nc.vector.tensor_copy(st[:], pt[:])
                else:
                    nc.scalar.copy(st[:], pt[:])
                nc.sync.dma_start(out_d[s * P : (s + 1) * P, :], st[:])

    nc.finalize()
    return nc


# ---------------------------------------------------------------- entry points

_CACHE = {}


def _get_program(inputs):
    node_ids = np.asarray(inputs["node_ids"])
    clique_ids = np.asarray(inputs["clique_ids"])
    inputs_arr = np.asarray(inputs["inputs"])
    N = int(inputs["nodes"])
    C = int(inputs["n_channels"])
    B, units_dim = inputs_arr.shape
    NC = units_dim // C

    key = (
        B,
        C,
        NC,
        N,
        node_ids.shape[0],
        hash(node_ids.tobytes()),
        hash(clique_ids.tobytes()),
    )
    if key not in _CACHE:
        plan = _plan(node_ids, clique_ids, N, NC, B, C)
        nc = _build(plan)
        _CACHE[key] = (plan, nc)
    return _CACHE[key]


def _run(inputs, trace=False):
    inputs_arr = np.asarray(inputs["inputs"]).astype(np.float32)
    N = int(inputs["nodes"])
    C = int(inputs["n_channels"])
    B = inputs_arr.shape[0]
    NC = inputs_arr.shape[1] // C
    bc = B * C

    plan, nc = _get_program(inputs)

    # host-side input staging: transpose to [NC, B*C] fp8-e3m4 (pure layout +
    # dtype cast; e3m4's 4 mantissa bits keep the deterministic rel err at
    # ~1.6e-2, under the 2e-2 gate)
    poolT = np.ascontiguousarray(
        inputs_arr.reshape(B, C, NC).transpose(2, 0, 1).reshape(NC, bc)
    ).astype(ml_dtypes.float8_e3m4)

    in_maps = []
    for d in range(N_CORES):
        ct = plan["cores"][d]
        in_maps.append(
            {
                "poolT": poolT,
                "idxtbl": ct["idx_tbl"],
                "htbl": ct["h_all"],
            }
        )

    res = run_bass_kernel_spmd(
        nc, in_maps, core_ids=list(range(N_CORES)), trace=trace
    )

    outT = np.empty((N, bc), np.float32)
    for d in range(N_CORES):
        o = np.asarray(res.results[d]["out"])  # [n_slots*128, bc] bf16
        ct = plan["cores"][d]
        for s, (g0, nn) in enumerate(ct["slot_meta"]):
            outT[g0 : g0 + nn] = o[s * P : s * P + nn].astype(np.float32)
    out = np.ascontiguousarray(
        outT.reshape(N, B, C).transpose(1, 2, 0)
    ).reshape(B, C * N)
    return out, res


def kernel(**inputs) -> np.ndarray:
    out, _ = _run(inputs, trace=False)
    return out


# revision 70
# speedup vs baseline: 1.0758x; 1.0758x over previous
"""GNN unpool (gather by clique id + scatter-add by node id) on 8 trn2 cores.

Problem: inputs [B=16, C*NC], node_ids/clique_ids [M], output [B, N*C] where
  pooled = inputs.reshape(B, C, NC)
  out[b, c, node_ids[m]] += pooled[b, c, clique_ids[m]]  for each m

Sharding: NODE ranges across 8 cores (each core owns ~N/8 nodes and the
~M/8 membership entries that target them). Every core holds the full
pooled tensor, staged by the host already transposed to poolT [NC, B*C]
fp8-e3m4, so the per-entry gather moves one 1KB row per entry (8x fewer
descriptors than batch sharding -> the SWDGE Q7 descriptor-generation
loop, ~8ns/index, and the sub-512B DMA penalty both drop ~8x; fp8
halves gather HBM traffic vs bf16 at a deterministic rel err of 1.6e-2
vs the 2e-2 gate).

Per-core device algorithm (uniform across cores; all per-core variation
lives in data tables so one SPMD program serves all 8):
  host packs the core's sorted entries into "slots": <=128 consecutive
  nodes and <=256 entries per slot -> exactly 2 chunks of 128 entry
  slots each (pad entries point at row 0 with an all-zero one-hot row);
  entries within a slot are ordered by clique id so gather reads are
  monotone in HBM address.
  1. dma_gather 1KB fp8 poolT rows for each chunk entry -> token layout
     ut[entry%128, chunk, B*C], groups alternating between two SWDGE
     queues so consecutive drains overlap
  2. one-hot H[entry, local-node] tables ([128, 128] fp8 per chunk) are
     compile-time known and staged from HBM directly (1.7MB) -- no
     on-device build, which kept DVE off the critical path and removed
     its SBUF-port contention with the Q7 descriptor-generation loop
  3. per slot: PE matmul psum[node 128, bc] += H_c^T @ U_c over the
     slot's 2 chunks (H is the stationary operand; 2 PSUM banks/slot)
  4. ACT/DVE (alternating per slot) evacuate psum -> bf16 staging,
     DMA -> outT[slot*128 rows, bc]
Host unshards: concatenate valid slot rows -> outT [N, B*C] -> final
[B, C*N] transpose (pure layout + dtype upcast).
"""

import math
import sys

import numpy as np

sys.path.insert(0, "/opt/trn_rl_repo")

import ml_dtypes  # noqa: E402

from concourse import bacc, mybir, tile  # noqa: E402
from concourse.bass_utils import run_bass_kernel_spmd  # noqa: E402

P = 128
N_CORES = 8
ENT_PER_SLOT = 256  # 2 chunks of 128
GSZ_SLOTS = 8  # slots per gather group (middle groups)


# ---------------------------------------------------------------- host planning


def _plan(node_ids, clique_ids, N, NC, B, C):
    node_ids = np.asarray(node_ids).astype(np.int64)
    clique_ids = np.asarray(clique_ids).astype(np.int64)
    M = node_ids.shape[0]
    bc = B * C

    order = np.argsort(node_ids, kind="stable")
    snode = node_ids[order]
    sclq = clique_ids[order]
    deg = np.bincount(node_ids, minlength=N)
    cum = np.cumsum(deg)

    # per-core contiguous node ranges, balanced by entry count
    bounds = [0]
    for d in range(1, N_CORES):
        n = int(np.searchsorted(cum, d * M / N_CORES))
        bounds.append(min(n + 1, N))
    bounds.append(N)

    cores = []
    for d in range(N_CORES):
        n0, n1 = bounds[d], bounds[d + 1]
        # greedy slots: <=128 nodes, <=256 entries, nodes atomic
        slots = []  # (g0, n_nodes, e_lo, e_hi) with e offsets into sorted arrays
        g0 = n0
        nodes_in = 0
        ents_in = 0
        for n in range(n0, n1):
            dn = int(deg[n])
            assert dn <= ENT_PER_SLOT
            if nodes_in + 1 > P or ents_in + dn > ENT_PER_SLOT:
                slots.append((g0, nodes_in, ents_in))
                g0 = n
                nodes_in, ents_in = 0, 0
            nodes_in += 1
            ents_in += dn
        if nodes_in:
            slots.append((g0, nodes_in, ents_in))
        cores.append((n0, n1, slots))

    n_slots = max(len(c[2]) for c in cores)
    n_chunks = 2 * n_slots

    core_tables = []
    for d in range(N_CORES):
        n0, n1, slots = cores[d]
        ent_clq = np.zeros(n_chunks * P, np.int16)
        nidrel_flat = np.full(n_chunks * P, -1.0, np.float32)
        e = int(np.searchsorted(snode, n0))
        slot_meta = []
        for s, (g0, nn, ne) in enumerate(slots):
            # order the slot's entries by clique id so the gather's HBM reads
            # are monotone in address (H absorbs any within-chunk permutation)
            sl_clq = sclq[e : e + ne]
            sl_rel = (snode[e : e + ne] - g0).astype(np.float32)
            o = np.argsort(sl_clq, kind="stable")
            idx = np.arange(ne)
            base = 2 * s * P
            ent_clq[base + idx] = sl_clq[o].astype(np.int16)
            nidrel_flat[base + idx] = sl_rel[o]
            slot_meta.append((g0, nn))
            e += ne
        # wrap by 16 partitions, replicate to 128 (dma_gather convention)
        wrapped = ent_clq.reshape(-1, 16).T  # [16, n_chunks*8]
        idx_tbl = np.ascontiguousarray(np.tile(wrapped, (8, 1)))
        # one-hot H staged directly as fp8 data (compile-time known):
        # H[entry e, chunk c, local node j] = 1 iff entry's node == slot
        # node j; layout [128 partitions, n_chunks*128 cols]
        h_all = np.zeros((P, n_chunks * P), ml_dtypes.float8_e3m4)
        pos = np.nonzero(nidrel_flat >= 0)[0]
        cc = pos // P
        ee = pos % P
        vv = nidrel_flat[pos].astype(np.int64)
        h_all[ee, cc * P + vv] = 1.0
        core_tables.append(
            dict(idx_tbl=idx_tbl, h_all=h_all, slot_meta=slot_meta, n0=n0, n1=n1)
        )

    return dict(
        M=M,
        N=N,
        NC=NC,
        bc=bc,
        n_slots=n_slots,
        n_chunks=n_chunks,
        cores=core_tables,
    )


# ---------------------------------------------------------------- device build


def _build(plan):
    NC = plan["NC"]
    bc = plan["bc"]
    n_slots = plan["n_slots"]
    n_chunks = plan["n_chunks"]
    assert bc % 1024 == 0 or bc in (512, 1024)
    half = bc // 2

    f32 = mybir.dt.float32
    bf16 = mybir.dt.bfloat16
    f16 = mybir.dt.float16
    f8 = mybir.dt.float8e3  # e3m4
    i16 = mybir.dt.int16

    # group schedule: small leading groups so the drain/compute pipeline
    # fills early, GSZ_SLOTS-sized middle, tapered tail so the final
    # drain+compute+write chain is short
    sizes = []
    for sz in (2, 4):
        if sum(sizes) + sz <= n_slots:
            sizes.append(sz)
    while n_slots - sum(sizes) > GSZ_SLOTS + 4:
        sizes.append(GSZ_SLOTS)
    rem = n_slots - sum(sizes)
    if rem > 4:
        sizes += [rem - 3, 2, 1]
    elif rem > 0:
        sizes.append(rem)
    group_bounds = []
    s = 0
    for z in sizes:
        group_bounds.append((s, s + z))
        s += z
    assert s == n_slots
    n_groups = len(group_bounds)

    nc = bacc.Bacc(None, target_bir_lowering=False, num_swdge_queues=2)

    poolT_d = nc.dram_tensor("poolT", [NC, bc], f8, kind="ExternalInput")
    idx_d = nc.dram_tensor("idxtbl", [P, n_chunks * 8], i16, kind="ExternalInput")
    h_d = nc.dram_tensor("htbl", [P, n_chunks * P], f8, kind="ExternalInput")
    out_d = nc.dram_tensor("out", [n_slots * P, bc], bf16, kind="ExternalOutput")

    with tile.TileContext(nc) as tc:
        with (
            tc.tile_pool(name="const", bufs=1) as constp,
            tc.tile_pool(name="upool", bufs=4) as upool,
            tc.tile_pool(name="opsum", bufs=4, space="PSUM") as opsum,
            tc.tile_pool(name="stage", bufs=6) as stagep,
        ):
            # load the first group's gather indices first so descgen can
            # start as early as possible, then H (one-hots), then the rest
            head_cols = 8 * 2 * sizes[0]
            h_head = P * 2 * sum(sizes[:2])
            idx_t = constp.tile([P, n_chunks * 8], i16)
            nc.sync.dma_start(idx_t[:, :head_cols], idx_d[:, :head_cols])
            h_t = constp.tile([P, n_chunks * P], f8)
            nc.sync.dma_start(h_t[:, :h_head], h_d[:, :h_head])
            nc.sync.dma_start(idx_t[:, head_cols:], idx_d[:, head_cols:])
            nc.sync.dma_start(h_t[:, h_head:], h_d[:, h_head:])

            u_tiles = {}

            def ensure_gather(g):
                if g in u_tiles or g >= n_groups:
                    return
                s0, s1 = group_bounds[g]
                nch = 2 * (s1 - s0)
                c0 = 2 * s0
                ut = upool.tile([P, 2 * GSZ_SLOTS, bc], f8, tag="utok")
                nidx = nch * P
                nc.gpsimd.dma_gather(
                    out_ap=ut[:, :nch, :],
                    in_ap=poolT_d[:],
                    idxs_ap=idx_t[:, c0 * 8 : (c0 + nch) * 8],
                    num_idxs=nidx,
                    num_idxs_reg=nidx,
                    elem_size=bc,
                    single_packet=False,
                    queue_num=g % 2,
                )
                u_tiles[g] = ut

            slot_group = {}
            for g, (s0, s1) in enumerate(group_bounds):
                for s in range(s0, s1):
                    slot_group[s] = g

            for s in range(n_slots):
                g = slot_group[s]
                ensure_gather(g)
                ensure_gather(g + 1)
                ut = u_tiles[g]
                la = 2 * (s - group_bounds[g][0])  # local chunk index in ut
                h0 = h_t[:, (2 * s) * P : (2 * s + 1) * P]
                h1 = h_t[:, (2 * s + 1) * P : (2 * s + 2) * P]

                pt = opsum.tile([P, bc], f32, tag="ps")  # 2 PSUM banks
                nc.tensor.matmul(
                    out=pt[:, :half],
                    lhsT=h0,
                    rhs=ut[:, la, :half],
                    start=True,
                    stop=False,
                )
                nc.tensor.matmul(
                    out=pt[:, half:],
                    lhsT=h0,
                    rhs=ut[:, la, half:],
                    start=True,
                    stop=False,
                )
                nc.tensor.matmul(
                    out=pt[:, :half],
                    lhsT=h1,
                    rhs=ut[:, la + 1, :half],
                    start=False,
                    stop=True,
                )
                nc.tensor.matmul(
                    out=pt[:, half:],
                    lhsT=h1,
                    rhs=ut[:, la + 1, half:],
                    start=False,
                    stop=True,
                )

                st = stagep.tile([P, bc], bf16, tag="st")
                # alternate evac engine: DVE is idle now that H comes from HBM
                if s % 2:
                    nc.vector.tensor_copy(st[:], pt[:])
                else:
                    nc.scalar.copy(st[:], pt[:])
                nc.sync.dma_start(out_d[s * P : (s + 1) * P, :], st[:])

    nc.finalize()
    return nc


# ---------------------------------------------------------------- entry points

_CACHE = {}


def _get_program(inputs):
    node_ids = np.asarray(inputs["node_ids"])
    clique_ids = np.asarray(inputs["clique_ids"])
    inputs_arr = np.asarray(inputs["inputs"])
    N = int(inputs["nodes"])
    C = int(inputs["n_channels"])
    B, units_dim = inputs_arr.shape
    NC = units_dim // C

    key = (
        B,
        C,
        NC,
        N,
        node_ids.shape[0],
        hash(node_ids.tobytes()),
        hash(clique_ids.tobytes()),
    )
    if key not in _CACHE:
        plan = _plan(node_ids, clique_ids, N, NC, B, C)
        nc = _build(plan)
        _CACHE[key] = (plan, nc)
    return _CACHE[key]


def _run(inputs, trace=False):
    inputs_arr = np.asarray(inputs["inputs"]).astype(np.float32)
    N = int(inputs["nodes"])
    C = int(inputs["n_channels"])
    B = inputs_arr.shape[0]
    NC = inputs_arr.shape[1] // C
    bc = B * C

    plan, nc = _get_program(inputs)

    # host-side input staging: transpose to [NC, B*C] fp8-e3m4 (pure layout +
    # dtype cast; e3m4's 4 mantissa bits keep the deterministic rel err at
    # ~1.6e-2, under the 2e-2 gate)
    poolT = np.ascontiguousarray(
        inputs_arr.reshape(B, C, NC).transpose(2, 0, 1).reshape(NC, bc)
    ).astype(ml_dtypes.float8_e3m4)

    in_maps = []
    for d in range(N_CORES):
        ct = plan["cores"][d]
        in_maps.append(
            {
                "poolT": poolT,
                "idxtbl": ct["idx_tbl"],
                "htbl": ct["h_all"],
            }
        )

    res = run_bass_kernel_spmd(
        nc, in_maps, core_ids=list(range(N_CORES)), trace=trace
    )

    outT = np.empty((N, bc), np.float32)
    for d in range(N_CORES):
        o = np.asarray(res.results[d]["out"])  # [n_slots*128, bc] bf16
        ct = plan["cores"][d]
        for s, (g0, nn) in enumerate(ct["slot_meta"]):
            outT[g0 : g0 + nn] = o[s * P : s * P + nn].astype(np.float32)
    out = np.ascontiguousarray(
        outT.reshape(N, B, C).transpose(1, 2, 0)
    ).reshape(B, C * N)
    return out, res


def kernel(**inputs) -> np.ndarray:
    out, _ = _run(inputs, trace=False)
    return out
